# revision 1
# baseline (speedup 1.0000x reference)
"""Trainium2 Bass kernel for nn_BusDecoder (moe_routing).

Computes out[b, n*2+o] = sum_d H[b,n,d] * W[t_n, d, o] + b[t_n, o] with
t_n = bus_type[0, n], for B=32, N=4096, D=1024, OUT=2, 3 types.

Strategy (memory-bound regime):
  - Data-parallel over batch B across 8 cores (B_local=4 per core).
  - Host splits H into f16 hi + f16 lo parts (Dekker-style) and transposes to
    feature-major [D, tokens]; the device matmuls recover fp32-level accuracy:
        out = Hhi@Whi + Hhi@Wlo + Hlo@Whi (+bias),  residual ~ Hlo@Wlo ~ 2^-22.
  - On device, one 12-wide weight stack [Whi | Wlo] is the stationary operand;
    Hhi streams once (produces both Whi and Wlo products), Hlo streams once
    against Whi accumulating into the same PSUM partitions 0:6.
  - Bias is added by a K=1 matmul against a ones vector.
  - Routing/selection on device: PSUM[12, F] * mask12 (one-hot by bus type,
    per token) on VectorE, then a K=12 matmul with a constant 0/1 matrix
    T[12, 2] sums the per-type pair into the final [2, F] output.
"""

import numpy as np

import concourse.bacc as bacc
import concourse.bass_utils as bass_utils
import concourse.mybir as mybir
import concourse.tile as tile

B, N, D, OUT = 32, 4096, 1024, 2
N_TYPES = 3
N_CORES = 8
BL = B // N_CORES          # 4 batch rows per core
TOK = BL * N               # 16384 tokens per core
P = 128
DCH = D // P               # 8 contraction chunks
CH = 1024                  # tokens per DMA chunk (2 MiB per f16 H chunk)
G = 512                    # tokens per matmul group (one PSUM bank of fp32)
C12 = 2 * N_TYPES * 2      # 12 = [Whi | Wlo] stacked output channels

USE_LO = True              # stream the f16 lo-part of H (fp32-level accuracy)

_CACHED_NC = None


def _build_nc():
    global _CACHED_NC
    if _CACHED_NC is not None:
        return _CACHED_NC

    f16 = mybir.dt.float16
    f32 = mybir.dt.float32

    nc = bacc.Bacc("TRN2", debug=False)
    hhi = nc.dram_tensor("hhi", [D, TOK], f16, kind="ExternalInput")
    if USE_LO:
        hlo = nc.dram_tensor("hlo", [D, TOK], f16, kind="ExternalInput")
    wstk = nc.dram_tensor("wstk", [D, C12], f16, kind="ExternalInput")
    bstk = nc.dram_tensor("bstk", [1, C12], f16, kind="ExternalInput")
    mask = nc.dram_tensor("mask12", [C12, TOK], f32, kind="ExternalInput")
    tmat = nc.dram_tensor("tmat", [C12, OUT], f32, kind="ExternalInput")
    out = nc.dram_tensor("out", [OUT, TOK], f32, kind="ExternalOutput")

    with tile.TileContext(nc) as tc:
        with (
            tc.tile_pool(name="const", bufs=1) as cp,
            tc.tile_pool(name="hp", bufs=3) as hp,
            tc.tile_pool(name="wk", bufs=3) as wk,
            tc.tile_pool(name="ps", bufs=3, space="PSUM") as ps,
            tc.tile_pool(name="ps2", bufs=3, space="PSUM") as ps2,
        ):
            wt = cp.tile([P, DCH, C12], f16, name="wt")
            nc.sync.dma_start(wt[:], wstk.ap().rearrange("(do p) c -> p do c", p=P))
            bt = cp.tile([1, C12], f16, name="bt")
            nc.sync.dma_start(bt[:], bstk.ap())
            tt = cp.tile([C12, OUT], f32, name="tt")
            nc.sync.dma_start(tt[:], tmat.ap())
            ones = cp.tile([1, G], f16, name="ones")
            nc.vector.memset(ones[:], 1.0)
            selbuf = cp.tile([OUT, TOK], f32, name="selbuf")

            hv = hhi.ap().rearrange("(do p) t -> p do t", p=P)
            if USE_LO:
                lv = hlo.ap().rearrange("(do p) t -> p do t", p=P)
            mv = mask.ap()

            for c in range(TOK // CH):
                csl = slice(c * CH, (c + 1) * CH)
                ht = hp.tile([P, DCH, CH], f16, name="ht")
                nc.sync.dma_start(ht[:], hv[:, :, csl])
                if USE_LO:
                    lt = hp.tile([P, DCH, CH], f16, name="lt")
                    nc.sync.dma_start(lt[:], lv[:, :, csl])
                mt = hp.tile([C12, CH], f32, name="mt")
                nc.sync.dma_start(mt[:], mv[:, csl])

                for g in range(CH // G):
                    gs = slice(g * G, (g + 1) * G)
                    p = ps.tile([C12, G], f32, name="p")
                    for do in range(DCH):
                        nc.tensor.matmul(
                            p[:], wt[:, do, :], ht[:, do, gs],
                            start=(do == 0), stop=False, skip_group_check=True,
                        )
                    if USE_LO:
                        for do in range(DCH):
                            nc.tensor.matmul(
                                p[0:6], wt[:, do, 0:6], lt[:, do, gs],
                                start=False, stop=False, skip_group_check=True,
                            )
                    nc.tensor.matmul(
                        p[:], bt[:], ones[:],
                        start=False, stop=True, skip_group_check=True,
                    )
                    m = wk.tile([C12, G], f32, name="m")
                    nc.vector.tensor_mul(m[:], p[:], mt[:, gs])
                    p2 = ps2.tile([OUT, G], f32, name="p2")
                    nc.tensor.matmul(
                        p2[:], tt[:], m[:],
                        start=True, stop=True, skip_group_check=True,
                    )
                    off = c * CH + g * G
                    nc.vector.tensor_copy(selbuf[:, off:off + G], p2[:])

            nc.sync.dma_start(out.ap(), selbuf[:])

    nc.compile()
    _CACHED_NC = nc
    return nc


def _host_prep(H, bus_type, W, b):
    """Shard + precision-split inputs; returns per-core in_maps."""
    H = np.asarray(H, dtype=np.float32)
    W = np.asarray(W, dtype=np.float32)
    b = np.asarray(b, dtype=np.float32)
    types = np.asarray(bus_type)[0].astype(np.int64)  # decoder choice = row 0

    # Weight stack [D, 12]: cols 2t+o = Whi[t,:,o], cols 6+2t+o = Wlo[t,:,o]
    W6 = np.ascontiguousarray(W.transpose(1, 0, 2).reshape(D, 2 * N_TYPES))
    Whi = W6.astype(np.float16)
    Wlo = (W6 - Whi.astype(np.float32)).astype(np.float16)
    wstk = np.ascontiguousarray(np.concatenate([Whi, Wlo], axis=1))

    b6 = b.reshape(2 * N_TYPES)
    bhi = b6.astype(np.float16)
    blo = (b6 - bhi.astype(np.float32)).astype(np.float16)
    bstk = np.concatenate([bhi, blo]).reshape(1, C12)

    # One-hot routing mask per token (token j = b_local*N + n -> depends on n)
    oh = (types[None, :] == np.arange(N_TYPES)[:, None])      # [3, N]
    m6 = np.repeat(oh, 2, axis=0)                             # [6, N]
    m6t = np.tile(m6, (1, BL))                                # [6, TOK]
    mask12 = np.ascontiguousarray(
        np.concatenate([m6t, m6t], axis=0).astype(np.float32)
    )

    # Constant pair-sum matrix: sel[o] = sum_{c: c%2==o} M[c]
    tmat = np.zeros((C12, OUT), np.float32)
    tmat[0::2, 0] = 1.0
    tmat[1::2, 1] = 1.0

    in_maps = []
    for ci in range(N_CORES):
        Hc = np.ascontiguousarray(H[ci * BL:(ci + 1) * BL].reshape(TOK, D).T)
        hhi = Hc.astype(np.float16)
        im = {
            "hhi": hhi,
            "wstk": wstk,
            "bstk": bstk,
            "mask12": mask12,
            "tmat": tmat,
        }
        if USE_LO:
            im["hlo"] = (Hc - hhi.astype(np.float32)).astype(np.float16)
        in_maps.append(im)
    return in_maps


def _unshard(results):
    outs = []
    for ci in range(N_CORES):
        ot = results[ci]["out"]  # [2, TOK] f32
        outs.append(ot.reshape(OUT, BL, N).transpose(1, 2, 0).reshape(BL, N * OUT))
    return np.ascontiguousarray(np.concatenate(outs, axis=0).astype(np.float32))


def kernel(H, bus_type, W, b):
    nc = _build_nc()
    in_maps = _host_prep(H, bus_type, W, b)
    res = bass_utils.run_bass_kernel_spmd(
        nc, in_maps, core_ids=list(range(N_CORES))
    )
    return _unshard(res.results)


if __name__ == "__main__":
    rng = np.random.default_rng(0)
    H = rng.standard_normal((B, N, D)).astype(np.float32)
    bus_type = rng.integers(0, N_TYPES, size=(B, N)).astype(np.int64)
    W = rng.uniform(-1 / 32, 1 / 32, size=(N_TYPES, D, OUT)).astype(np.float32)
    b = rng.uniform(-1 / 32, 1 / 32, size=(N_TYPES, OUT)).astype(np.float32)
    got = kernel(H, bus_type, W, b)
    types = bus_type[0]
    Wn = W[types]
    bn = b[types]
    want = (np.einsum("bnd,ndo->bno", H, Wn) + bn[None]).reshape(B, -1)
    err = np.abs(got - want)
    rel = err.max() / np.abs(want).max()
    print("max abs err:", err.max(), "absmax-rel:", rel)
    print("rms rel:", np.sqrt((err**2).mean() / (want**2).mean()))


# revision 5
# speedup vs baseline: 21.1431x; 21.1431x over previous
"""Trainium2 Bass kernel for nn_BusDecoder (moe_routing).

Computes out[b, n*2+o] = sum_d H[b,n,d] * W[t_n, d, o] + b[t_n, o] with
t_n = bus_type[0, n], for B=32, N=4096, D=1024, OUT=2, 3 types.

Strategy (memory-bound regime):
  - Data-parallel over batch B across 8 cores (B_local=4 per core).
  - Host splits H into f16 hi + f16 lo parts (Dekker-style) and transposes to
    feature-major [D, tokens]; the device matmuls recover fp32-level accuracy:
        out = Hhi@Whi + Hhi@Wlo + Hlo@Whi (+bias),  residual ~ Hlo@Wlo ~ 2^-22.
  - On device, one 12-wide weight stack [Whi | Wlo] is the stationary operand;
    Hhi streams once (produces both Whi and Wlo products), Hlo streams once
    against Whi accumulating into the same PSUM partitions 0:6.
  - Bias is added by a K=1 matmul against a ones vector.
  - Routing/selection on device: PSUM[12, F] * mask12 (one-hot by bus type,
    per token) on VectorE, then a K=12 matmul with a constant 0/1 matrix
    T[12, 2] sums the per-type pair into the final [2, F] output.
"""

import numpy as np

import concourse.bacc as bacc
import concourse.bass_utils as bass_utils
import concourse.mybir as mybir
import concourse.tile as tile

B, N, D, OUT = 32, 4096, 1024, 2
N_TYPES = 3
N_CORES = 8
BL = B // N_CORES          # 4 batch rows per core
TOK = BL * N               # 16384 tokens per core
P = 128
DCH = D // P               # 8 contraction chunks
CH = 1024                  # tokens per DMA chunk (2 MiB per f16 H chunk)
G = 512                    # tokens per matmul group (one PSUM bank of fp32)
C12 = 2 * N_TYPES * 2      # 12 = [Whi | Wlo] stacked output channels

USE_LO = True              # stream the f16 lo-part of H (fp32-level accuracy)

_CACHED_NC = {}


def _build_nc(repeat=1):
    # repeat>1 wraps the body in a device-side For_i loop running the
    # identical workload `repeat` times — used only by test.py to measure
    # per-execution hardware time through the high-latency axon tunnel.
    if repeat in _CACHED_NC:
        return _CACHED_NC[repeat]

    f16 = mybir.dt.float16
    f32 = mybir.dt.float32

    nc = bacc.Bacc("TRN2", debug=False)
    hhi = nc.dram_tensor("hhi", [D, TOK], f16, kind="ExternalInput")
    if USE_LO:
        hlo = nc.dram_tensor("hlo", [D, TOK], f16, kind="ExternalInput")
    wstk = nc.dram_tensor("wstk", [D, C12], f16, kind="ExternalInput")
    bstk = nc.dram_tensor("bstk", [1, C12], f16, kind="ExternalInput")
    mask = nc.dram_tensor("mask12", [C12, TOK], f32, kind="ExternalInput")
    tmat = nc.dram_tensor("tmat", [C12, OUT], f32, kind="ExternalInput")
    out = nc.dram_tensor("out", [OUT, TOK], f32, kind="ExternalOutput")

    with tile.TileContext(nc) as tc:
        with (
            tc.tile_pool(name="const", bufs=1) as cp,
            tc.tile_pool(name="hp", bufs=3) as hp,
            tc.tile_pool(name="wk", bufs=3) as wk,
            tc.tile_pool(name="ps", bufs=3, space="PSUM") as ps,
            tc.tile_pool(name="ps2", bufs=3, space="PSUM") as ps2,
        ):
            wt = cp.tile([P, DCH, C12], f16, name="wt")
            nc.sync.dma_start(wt[:], wstk.ap().rearrange("(do p) c -> p do c", p=P))
            bt = cp.tile([1, C12], f16, name="bt")
            nc.sync.dma_start(bt[:], bstk.ap())
            tt = cp.tile([C12, OUT], f32, name="tt")
            nc.sync.dma_start(tt[:], tmat.ap())
            ones = cp.tile([1, G], f16, name="ones")
            nc.vector.memset(ones[:], 1.0)
            selbuf = cp.tile([OUT, TOK], f32, name="selbuf")

            hv = hhi.ap().rearrange("(do p) t -> p do t", p=P)
            lv = hlo.ap().rearrange("(do p) t -> p do t", p=P) if USE_LO else None
            mv = mask.ap()

            def body():
                _emit_body(nc, hv, lv, mv, out, hp, wk, ps, ps2,
                           wt, bt, tt, ones, selbuf)

            if repeat == 1:
                body()
            else:
                with tc.For_i(0, repeat, 1):
                    body()

    nc.compile()
    _CACHED_NC[repeat] = nc
    return nc


def _emit_body(nc, hv, lv, mv, out, hp, wk, ps, ps2, wt, bt, tt, ones, selbuf):
    f16 = mybir.dt.float16
    f32 = mybir.dt.float32
    for c in range(TOK // CH):
        csl = slice(c * CH, (c + 1) * CH)
        ht = hp.tile([P, DCH, CH], f16, name="ht")
        nc.sync.dma_start(ht[:], hv[:, :, csl])
        if USE_LO:
            lt = hp.tile([P, DCH, CH], f16, name="lt")
            nc.sync.dma_start(lt[:], lv[:, :, csl])
        mt = hp.tile([C12, CH], f32, name="mt")
        nc.sync.dma_start(mt[:], mv[:, csl])

        for g in range(CH // G):
            gs = slice(g * G, (g + 1) * G)
            p = ps.tile([C12, G], f32, name="p")
            for do in range(DCH):
                nc.tensor.matmul(
                    p[:], wt[:, do, :], ht[:, do, gs],
                    start=(do == 0), stop=False, skip_group_check=True,
                )
            if USE_LO:
                for do in range(DCH):
                    nc.tensor.matmul(
                        p[0:6], wt[:, do, 0:6], lt[:, do, gs],
                        start=False, stop=False, skip_group_check=True,
                    )
            nc.tensor.matmul(
                p[:], bt[:], ones[:],
                start=False, stop=True, skip_group_check=True,
            )
            m = wk.tile([C12, G], f32, name="m")
            nc.vector.tensor_mul(m[:], p[:], mt[:, gs])
            p2 = ps2.tile([OUT, G], f32, name="p2")
            nc.tensor.matmul(
                p2[:], tt[:], m[:],
                start=True, stop=True, skip_group_check=True,
            )
            off = c * CH + g * G
            nc.vector.tensor_copy(selbuf[:, off:off + G], p2[:])

    nc.sync.dma_start(out.ap(), selbuf[:])


def _host_prep(H, bus_type, W, b):
    """Shard + precision-split inputs; returns per-core in_maps."""
    H = np.asarray(H, dtype=np.float32)
    W = np.asarray(W, dtype=np.float32)
    b = np.asarray(b, dtype=np.float32)
    types = np.asarray(bus_type)[0].astype(np.int64)  # decoder choice = row 0

    # Weight stack [D, 12]: cols 2t+o = Whi[t,:,o], cols 6+2t+o = Wlo[t,:,o]
    W6 = np.ascontiguousarray(W.transpose(1, 0, 2).reshape(D, 2 * N_TYPES))
    Whi = W6.astype(np.float16)
    Wlo = (W6 - Whi.astype(np.float32)).astype(np.float16)
    wstk = np.ascontiguousarray(np.concatenate([Whi, Wlo], axis=1))

    b6 = b.reshape(2 * N_TYPES)
    bhi = b6.astype(np.float16)
    blo = (b6 - bhi.astype(np.float32)).astype(np.float16)
    bstk = np.concatenate([bhi, blo]).reshape(1, C12)

    # One-hot routing mask per token (token j = b_local*N + n -> depends on n)
    oh = (types[None, :] == np.arange(N_TYPES)[:, None])      # [3, N]
    m6 = np.repeat(oh, 2, axis=0)                             # [6, N]
    m6t = np.tile(m6, (1, BL))                                # [6, TOK]
    mask12 = np.ascontiguousarray(
        np.concatenate([m6t, m6t], axis=0).astype(np.float32)
    )

    # Constant pair-sum matrix: sel[o] = sum_{c: c%2==o} M[c]
    tmat = np.zeros((C12, OUT), np.float32)
    tmat[0::2, 0] = 1.0
    tmat[1::2, 1] = 1.0

    in_maps = []
    for ci in range(N_CORES):
        Hc = np.ascontiguousarray(H[ci * BL:(ci + 1) * BL].reshape(TOK, D).T)
        hhi = Hc.astype(np.float16)
        im = {
            "hhi": hhi,
            "wstk": wstk,
            "bstk": bstk,
            "mask12": mask12,
            "tmat": tmat,
        }
        if USE_LO:
            im["hlo"] = (Hc - hhi.astype(np.float32)).astype(np.float16)
        in_maps.append(im)
    return in_maps


def _unshard(results):
    outs = []
    for ci in range(N_CORES):
        ot = results[ci]["out"]  # [2, TOK] f32
        outs.append(ot.reshape(OUT, BL, N).transpose(1, 2, 0).reshape(BL, N * OUT))
    return np.ascontiguousarray(np.concatenate(outs, axis=0).astype(np.float32))


def kernel(H, bus_type, W, b):
    nc = _build_nc()
    in_maps = _host_prep(H, bus_type, W, b)
    res = bass_utils.run_bass_kernel_spmd(
        nc, in_maps, core_ids=list(range(N_CORES))
    )
    return _unshard(res.results)


if __name__ == "__main__":
    rng = np.random.default_rng(0)
    H = rng.standard_normal((B, N, D)).astype(np.float32)
    bus_type = rng.integers(0, N_TYPES, size=(B, N)).astype(np.int64)
    W = rng.uniform(-1 / 32, 1 / 32, size=(N_TYPES, D, OUT)).astype(np.float32)
    b = rng.uniform(-1 / 32, 1 / 32, size=(N_TYPES, OUT)).astype(np.float32)
    got = kernel(H, bus_type, W, b)
    types = bus_type[0]
    Wn = W[types]
    bn = b[types]
    want = (np.einsum("bnd,ndo->bno", H, Wn) + bn[None]).reshape(B, -1)
    err = np.abs(got - want)
    rel = err.max() / np.abs(want).max()
    print("max abs err:", err.max(), "absmax-rel:", rel)
    print("rms rel:", np.sqrt((err**2).mean() / (want**2).mean()))
